# revision 9
# baseline (speedup 1.0000x reference)
"""Trainium2 Bass kernel for nn_Downsample_Spa: sigma-conv + gaussian unfold downsample.

Math (per batch image, all on one NeuronCore; batch of 8 -> 8 cores):
  xp = reflect_pad(x)                                  # [64,130,130]
  sigma[o,p] = clamp(BN(conv3x3(xp))[o,p], 1e-4)       # only needed at stride-2 positions p
  graw[o,p]  = exp(-0.5*d2[o]/sigma^2) / sigma         # (sqrt(2pi) cancels in normalization)
  gn[o,p]    = graw[o,p] / sum_o' graw[o',p]
  out[c,p]   = sum_o gn[o,p] * xp[c, p + offset(o)]

Device layout: partitions = (row-half hh, channel c) = 128. Host prepends
reflect padding so every tap is a clean strided AP. Conv runs as 9 accumulating
fp32r matmuls with block-diagonal weights (M=18 covers both row halves in one
N-stream). g pipeline runs on ACT via exp/ln (no reciprocal needed). The
per-position g is broadcast across the 64 channel partitions with a one-hot PE
matmul, then DVE multiplies and accumulates the 9 taps.
"""

import os
import sys

import numpy as np

if "/opt/trn_rl_repo" not in sys.path:
    sys.path.insert(0, "/opt/trn_rl_repo")

K = 3
BN_EPS = 1e-5
SIGMA_MIN = 1e-4
N, C, H, W = 8, 64, 128, 128
HO = WO = 64            # output spatial dims
HH = 2                  # row halves
RS = 65                 # padded-row slots per partition
WS = 130                # padded-col slots
HOC = 32                # output rows per half
NBLK = 4                # position blocks (8 output-row-pairs each)
BR = HOC // NBLK        # ho' rows per block = 8
NPOS = BR * WO          # matmul N per block = 512

# consts tensor column layout
_W0 = 0                  # Wblk: 9 taps x 18 cols
_OS = _W0 + 9 * 18       # onesS [18,2]
_OR = _OS + 2            # onesR [2,128]
_OG = _OR + 128          # onesG: 9 taps x 128 cols
_D2 = _OG + 9 * 128      # d2 scale [18,1]
_BC = _D2 + 1            # bias-clamp [18,1]
_EP = _BC + 1            # sigma_min const [18,1]
_NCC = _EP + 1

_STATE = {}


def _build_consts(conv_w, bn_gamma, bn_beta, bn_mean, bn_var):
    s = (bn_gamma / np.sqrt(bn_var + BN_EPS)).astype(np.float32)      # [9]
    wf = conv_w.astype(np.float32) * s[:, None, None, None]           # [9,64,3,3]
    bias = (bn_beta - bn_mean * s).astype(np.float32)                 # [9]

    cst = np.zeros((128, _NCC), np.float32)
    for tap in range(9):
        i, j = tap // 3, tap % 3
        for hh in range(HH):
            # lhsT column m = hh*9 + o ; row k = hh*64 + c
            cst[hh * 64:hh * 64 + 64, _W0 + tap * 18 + hh * 9:_W0 + tap * 18 + hh * 9 + 9] = \
                wf[:, :, i, j].T  # [c, o]
    for hh in range(HH):
        cst[hh * 9:hh * 9 + 9, _OS + hh] = 1.0                        # onesS [18,2]
        cst[hh, _OR + hh * 64:_OR + hh * 64 + 64] = 1.0              # onesR [2,128]
        for tap in range(9):
            cst[hh * 9 + tap, _OG + tap * 128 + hh * 64:_OG + tap * 128 + hh * 64 + 64] = 1.0
    d2 = np.array([(kk // 3 - 1) ** 2 + (kk % 3 - 1) ** 2 for kk in range(9)], np.float32)
    for hh in range(HH):
        cst[hh * 9:hh * 9 + 9, _D2] = -0.5 * d2
        cst[hh * 9:hh * 9 + 9, _BC] = bias - SIGMA_MIN
        cst[hh * 9:hh * 9 + 9, _EP] = SIGMA_MIN
    return cst


def _build_bass(for_sim=False):
    import concourse.bass as bass
    import concourse.tile as tile
    from concourse import mybir

    f32 = mybir.dt.float32
    f32r = mybir.dt.float32r
    MULT = mybir.AluOpType.mult
    ADD = mybir.AluOpType.add
    MAX = mybir.AluOpType.max
    AF = mybir.ActivationFunctionType

    if for_sim:
        nc = bass.Bass("TRN2", target_bir_lowering=False, detect_race_conditions=False)
    else:
        from concourse import bacc
        nc = bacc.Bacc()
    xin = nc.dram_tensor("xin", [128, RS, WS], f32r, kind="ExternalInput")
    cin = nc.dram_tensor("cin", [128, _NCC], f32r, kind="ExternalInput")
    out = nc.dram_tensor("out", [128, HOC, WO], f32, kind="ExternalOutput")

    with tile.TileContext(nc) as tc:
        from contextlib import ExitStack
        with ExitStack() as ctx:
            big = ctx.enter_context(tc.tile_pool(name="big", bufs=1))
            gsb = ctx.enter_context(tc.tile_pool(name="gsb", bufs=2))
            acc_p = ctx.enter_context(tc.tile_pool(name="acc", bufs=2))
            y_p = ctx.enter_context(tc.tile_pool(name="y", bufs=2))
            ps_s = ctx.enter_context(tc.tile_pool(name="ps_s", bufs=2, space="PSUM"))
            ps_m = ctx.enter_context(tc.tile_pool(name="ps_m", bufs=2, space="PSUM"))
            ps_m2 = ctx.enter_context(tc.tile_pool(name="ps_m2", bufs=2, space="PSUM"))
            ps_g = ctx.enter_context(tc.tile_pool(name="ps_g", bufs=2, space="PSUM"))

            xs = big.tile([128, RS, WS], f32r)
            cs = big.tile([128, _NCC], f32r)
            nc.sync.dma_start(out=xs[:], in_=xin[:])
            nc.sync.dma_start(out=cs[:], in_=cin[:])

            def xtap(tap, blk):
                i, j = tap // 3, tap % 3
                r0 = 2 * BR * blk + i
                return xs[:, r0:r0 + 2 * BR - 1:2, j:j + 2 * WO - 1:2]  # [128, 8, 64]

            def conv_emit(blk):
                sig = ps_s.tile([18, NPOS], f32, tag="sig")
                for tap in range(9):
                    nc.tensor.matmul(
                        sig[:],
                        cs[:, _W0 + tap * 18:_W0 + (tap + 1) * 18],
                        xtap(tap, blk),
                        start=(tap == 0), stop=(tap == 8),
                    )
                return sig

            def block_emit(blk, sig):
                # ---- g pipeline: graw = exp(-0.5*d2/sig^2)/sig (unnormalized) ----
                sc = gsb.tile([18, NPOS], f32, tag="sc")
                nc.vector.tensor_scalar(out=sc[:], in0=sig[:],
                                        scalar1=cs[0:18, _BC:_BC + 1].bitcast(f32),
                                        scalar2=float(SIGMA_MIN),
                                        op0=ADD, op1=MAX)
                inv = gsb.tile([18, NPOS], f32, tag="inv")
                nc.vector.reciprocal_approx_fast(out=inv[:], in_=sc[:])
                qt = gsb.tile([18, NPOS], f32, tag="qt")
                nc.scalar.activation(out=qt[:], in_=inv[:], func=AF.Square)
                et = gsb.tile([18, NPOS], f32, tag="et")
                nc.scalar.activation(out=et[:], in_=qt[:], func=AF.Exp,
                                     scale=cs[0:18, _D2:_D2 + 1].bitcast(f32))
                graw = gsb.tile([18, NPOS], f32r, tag="graw")
                nc.vector.tensor_tensor(out=graw[:], in0=et[:], in1=inv[:], op=MULT)

                # ---- normalizer path (hidden behind the unfold) ----
                S = ps_m.tile([2, NPOS], f32, tag="S")
                nc.tensor.matmul(S[:], cs[0:18, _OS:_OS + 2],
                                 graw[:], start=True, stop=True)
                Sc = gsb.tile([2, NPOS], f32r, tag="Sc")
                nc.vector.tensor_copy(out=Sc[:], in_=S[:])
                Srep = ps_m2.tile([128, NPOS], f32, tag="Srep")
                nc.tensor.matmul(Srep[:], cs[0:2, _OR:_OR + 128],
                                 Sc[:], start=True, stop=True)
                rr = gsb.tile([128, NPOS], f32, tag="rr")
                nc.vector.reciprocal_approx_fast(out=rr[:], in_=Srep[:])

                # ---- unfold: yt[tap] = grep(tap) * xtap, tree-sum, scale ----
                yt = y_p.tile([128, 9, BR, WO], f32, tag="yt")
                for tap in range(9):
                    grep = ps_g.tile([128, NPOS], f32, tag="grep")
                    nc.tensor.matmul(grep[:], cs[0:18, _OG + tap * 128:_OG + (tap + 1) * 128],
                                     graw[:], start=True, stop=True)
                    nc.vector.tensor_tensor(out=yt[:, tap], in0=xtap(tap, blk).bitcast(f32),
                                            in1=grep[:], op=MULT)
                t4 = y_p.tile([128, 4, BR, WO], f32, tag="t4")
                nc.gpsimd.tensor_tensor(out=t4[:], in0=yt[:, 0:8:2], in1=yt[:, 1:8:2], op=ADD)
                t2 = y_p.tile([128, 2, BR, WO], f32, tag="t2")
                nc.gpsimd.tensor_tensor(out=t2[:], in0=t4[:, 0:4:2], in1=t4[:, 1:4:2], op=ADD)
                t1 = y_p.tile([128, BR, WO], f32, tag="t1")
                nc.vector.tensor_tensor(out=t1[:], in0=t2[:, 0], in1=t2[:, 1], op=ADD)
                t0 = y_p.tile([128, BR, WO], f32, tag="t0")
                nc.vector.tensor_tensor(out=t0[:], in0=t1[:], in1=yt[:, 8], op=ADD)
                acc = acc_p.tile([128, BR, WO], f32, tag="acc")
                nc.gpsimd.tensor_tensor(out=acc[:], in0=t0[:], in1=rr[:], op=MULT)
                nc.sync.dma_start(out=out[:, BR * blk:BR * (blk + 1), :], in_=acc[:])

            sigs = {0: conv_emit(0)}
            for blk in range(NBLK):
                if blk + 1 < NBLK:
                    sigs[blk + 1] = conv_emit(blk + 1)
                block_emit(blk, sigs.pop(blk))

    if not for_sim and not nc.is_finalized():
        nc.finalize()
    return nc


def _prep_inputs(x, conv_w, bn_gamma, bn_beta, bn_mean, bn_var):
    cst = _build_consts(conv_w, bn_gamma, bn_beta, bn_mean, bn_var)
    xp = np.pad(np.asarray(x, np.float32), ((0, 0), (0, 0), (1, 1), (1, 1)), mode="reflect")
    in_maps = []
    for n in range(N):
        xc = np.concatenate([xp[n, :, 0:RS, :], xp[n, :, 64:64 + RS, :]], axis=0)
        in_maps.append({"xin": np.ascontiguousarray(xc), "cin": cst})
    return in_maps


def _gather(results):
    out = np.empty((N, C, HO, WO), np.float32)
    for n in range(N):
        d = results[n]["out"]
        out[n, :, 0:HOC, :] = d[0:64]
        out[n, :, HOC:, :] = d[64:128]
    return out


def _enable_axon_trace():
    """Register the NTFF profile hook that this image's antenv lacks."""
    if _STATE.get("trace_hooked"):
        return
    import types
    import antenv
    from concourse import bass_utils
    mod = types.ModuleType("antenv.axon_hooks")
    mod._hook = None
    mod.set_axon_ntff_profile_hook = lambda h: setattr(mod, "_hook", h)
    mod.get_axon_ntff_profile_hook = lambda: mod._hook
    sys.modules["antenv.axon_hooks"] = mod
    antenv.axon_hooks = mod
    from trn_agent_boot.trn_boot import _ntff_profile_via_ctypes
    mod._hook = _ntff_profile_via_ctypes("/opt/axon/libaxon_pjrt.so")
    bass_utils.upload_artifacts = lambda tmpdir: tmpdir
    _STATE["trace_hooked"] = True


def run(x, conv_w, bn_gamma, bn_beta, bn_mean, bn_var, trace=False):
    from concourse.bass_utils import run_bass_kernel_spmd
    if trace:
        _enable_axon_trace()
    if "nc" not in _STATE:
        _STATE["nc"] = _build_bass()
    in_maps = _prep_inputs(x, conv_w, bn_gamma, bn_beta, bn_mean, bn_var)
    res = run_bass_kernel_spmd(_STATE["nc"], in_maps, list(range(N)), trace=trace)
    _STATE["last"] = res
    return _gather(res.results)


def kernel(x, conv_w, bn_gamma, bn_beta, bn_mean, bn_var):
    return run(x, conv_w, bn_gamma, bn_beta, bn_mean, bn_var,
               trace=bool(int(os.environ.get("KERNEL_TRACE", "0"))))


# revision 15
# speedup vs baseline: 1.1494x; 1.1494x over previous
"""Trainium2 Bass kernel for nn_Downsample_Spa: sigma-conv + gaussian unfold downsample.

Math (per batch image, all on one NeuronCore; batch of 8 -> 8 cores):
  xp = reflect_pad(x)                                  # [64,130,130]
  sigma[o,p] = clamp(BN(conv3x3(xp))[o,p], 1e-4)       # only needed at stride-2 positions p
  graw[o,p]  = exp(-0.5*d2[o]/sigma^2) / sigma         # (sqrt(2pi) cancels in normalization)
  gn[o,p]    = graw[o,p] / sum_o' graw[o',p]
  out[c,p]   = sum_o gn[o,p] * xp[c, p + offset(o)]

Device layout: partitions = (row-half hh, channel c) = 128. Host prepends
reflect padding so every tap is a clean strided AP. Conv runs as 9 accumulating
fp32r matmuls with block-diagonal weights (M=18 covers both row halves in one
N-stream). g pipeline runs on ACT via exp/ln (no reciprocal needed). The
per-position g is broadcast across the 64 channel partitions with a one-hot PE
matmul, then DVE multiplies and accumulates the 9 taps.
"""

import os
import sys

import numpy as np

if "/opt/trn_rl_repo" not in sys.path:
    sys.path.insert(0, "/opt/trn_rl_repo")

K = 3
BN_EPS = 1e-5
SIGMA_MIN = 1e-4
N, C, H, W = 8, 64, 128, 128
HO = WO = 64            # output spatial dims
HH = 2                  # row halves
RS = 65                 # padded-row slots per partition
WS = 130                # padded-col slots
HOC = 32                # output rows per half
NBLK = 4                # position blocks (8 output-row-pairs each)
BR = HOC // NBLK        # ho' rows per block = 8
NPOS = BR * WO          # matmul N per block = 512

# consts tensor column layout
_W0 = 0                  # Wblk: 9 taps x 18 cols
_OS = _W0 + 9 * 18       # onesS [18,2]
_OR = _OS + 2            # onesR [2,128]
_OG = _OR + 128          # onesG: 9 taps x 128 cols
_D2 = _OG + 9 * 128      # d2 scale [18,1]
_BC = _D2 + 1            # bias-clamp [18,1]
_EP = _BC + 1            # sigma_min const [18,1]
_NCC = _EP + 1

_STATE = {}


def _build_consts(conv_w, bn_gamma, bn_beta, bn_mean, bn_var):
    s = (bn_gamma / np.sqrt(bn_var + BN_EPS)).astype(np.float32)      # [9]
    wf = conv_w.astype(np.float32) * s[:, None, None, None]           # [9,64,3,3]
    bias = (bn_beta - bn_mean * s).astype(np.float32)                 # [9]

    cst = np.zeros((128, _NCC), np.float32)
    for tap in range(9):
        i, j = tap // 3, tap % 3
        for hh in range(HH):
            # lhsT column m = hh*9 + o ; row k = hh*64 + c
            cst[hh * 64:hh * 64 + 64, _W0 + tap * 18 + hh * 9:_W0 + tap * 18 + hh * 9 + 9] = \
                wf[:, :, i, j].T  # [c, o]
    for hh in range(HH):
        cst[hh * 9:hh * 9 + 9, _OS + hh] = 1.0                        # onesS [18,2]
        cst[hh, _OR + hh * 64:_OR + hh * 64 + 64] = 1.0              # onesR [2,128]
        for tap in range(9):
            cst[hh * 9 + tap, _OG + tap * 128 + hh * 64:_OG + tap * 128 + hh * 64 + 64] = 1.0
    d2 = np.array([(kk // 3 - 1) ** 2 + (kk % 3 - 1) ** 2 for kk in range(9)], np.float32)
    for hh in range(HH):
        cst[hh * 9:hh * 9 + 9, _D2] = -0.5 * d2
        cst[hh * 9:hh * 9 + 9, _BC] = bias - SIGMA_MIN
        cst[hh * 9:hh * 9 + 9, _EP] = SIGMA_MIN
    return cst


def _build_bass(for_sim=False):
    import concourse.bass as bass
    import concourse.tile as tile
    from concourse import mybir

    f32 = mybir.dt.float32
    f32r = mybir.dt.float32r
    bf16 = mybir.dt.bfloat16
    MULT = mybir.AluOpType.mult
    ADD = mybir.AluOpType.add
    MAX = mybir.AluOpType.max
    AF = mybir.ActivationFunctionType

    if for_sim:
        nc = bass.Bass("TRN2", target_bir_lowering=False, detect_race_conditions=False)
    else:
        from concourse import bacc
        nc = bacc.Bacc()
    xin = nc.dram_tensor("xin", [128, RS, WS], f32r, kind="ExternalInput")
    cin = nc.dram_tensor("cin", [128, _NCC], f32r, kind="ExternalInput")
    gin = nc.dram_tensor("gin", [18, 9 * 128 + 2], bf16, kind="ExternalInput")
    out = nc.dram_tensor("out", [128, HOC, WO], f32, kind="ExternalOutput")

    CR = 17                  # rows per chunk tile (16 + 1 overlap)
    JW = 66                  # parity-plane j slots (65 used + pad)

    with tile.TileContext(nc) as tc:
        from contextlib import ExitStack
        with ExitStack() as ctx:
            big = ctx.enter_context(tc.tile_pool(name="big", bufs=1))
            gsb = ctx.enter_context(tc.tile_pool(name="gsb", bufs=2))
            y_p = ctx.enter_context(tc.tile_pool(name="y", bufs=2))
            ps_s = ctx.enter_context(tc.tile_pool(name="ps_s", bufs=2, space="PSUM"))
            ps_m = ctx.enter_context(tc.tile_pool(name="ps_m", bufs=2, space="PSUM"))
            ps_m2 = ctx.enter_context(tc.tile_pool(name="ps_m2", bufs=2, space="PSUM"))
            ps_g = ctx.enter_context(tc.tile_pool(name="ps_g", bufs=2, space="PSUM"))

            cs = big.tile([128, _NCC], f32r)
            nc.sync.dma_start(out=cs[:], in_=cin[:])
            gs = big.tile([18, 9 * 128 + 2], bf16)
            nc.sync.dma_start(out=gs[:], in_=gin[:])

            # chunked input: per block a [128, 17, 130] f32r tile (+1 row overlap)
            xsk = []
            xbk = []   # bf16 parity-split copies: [128, 17, 2(par), JW] + dual-aligned even plane
            for blk in range(NBLK):
                xs = big.tile([128, CR, WS], f32r, tag=f"xs{blk}")
                nc.sync.dma_start(out=xs[:], in_=xin[:, 16 * blk:16 * blk + CR, :])
                xsk.append(xs)
                # bf16 copies: [r, 0, j] = even cols (w=2j), [r, 1, j] = odd cols (w=2j+1)
                # plus [r, 2, j] = even cols shifted (w=2j+2) for the 4B-aligned b=+1 path
                xb = big.tile([128, CR, 3, JW], bf16, tag=f"xb{blk}")
                nc.scalar.activation(out=xb[:, :, 0, 0:65], in_=xs[:, :, 0:130:2].bitcast(f32), func=AF.Copy)
                nc.scalar.activation(out=xb[:, :, 1, 0:65], in_=xs[:, :, 1:130:2].bitcast(f32), func=AF.Copy)
                nc.scalar.activation(out=xb[:, :, 2, 0:64], in_=xs[:, :, 2:130:2].bitcast(f32), func=AF.Copy)
                xbk.append(xb)

            def xtap(tap, blk):
                # f32r strided view for the conv rhs
                i, j = tap // 3, tap % 3
                return xsk[blk][:, i:i + 2 * BR - 1:2, j:j + 2 * WO - 1:2]  # [128, 8, 64]

            def xtap_bf(tap, blk):
                # bf16 step-1 view for the unfold mults
                i, b = tap // 3, tap % 3
                if b == 0:
                    pl, j0 = 0, 0
                elif b == 1:
                    pl, j0 = 1, 0
                else:
                    pl, j0 = 2, 0
                return xbk[blk][:, i:i + 2 * BR - 1:2, pl, j0:j0 + WO]  # [128, 8, 64]

            def conv_emit(blk):
                sig = ps_s.tile([18, NPOS], f32, tag="sig")
                for tap in range(9):
                    nc.tensor.matmul(
                        sig[:],
                        cs[:, _W0 + tap * 18:_W0 + (tap + 1) * 18],
                        xtap(tap, blk),
                        start=(tap == 0), stop=(tap == 8),
                    )
                return sig

            def block_emit(blk, sig):
                # ---- g pipeline: graw = exp(-0.5*d2/sig^2)/sig (unnormalized) ----
                sc = gsb.tile([18, NPOS], f32, tag="sc")
                nc.vector.tensor_scalar(out=sc[:], in0=sig[:],
                                        scalar1=cs[0:18, _BC:_BC + 1].bitcast(f32),
                                        scalar2=float(SIGMA_MIN),
                                        op0=ADD, op1=MAX)
                inv = gsb.tile([18, NPOS], f32, tag="inv")
                nc.vector.reciprocal_approx_fast(out=inv[:], in_=sc[:])
                qt = gsb.tile([18, NPOS], f32, tag="qt")
                nc.scalar.activation(out=qt[:], in_=inv[:], func=AF.Square)
                et = gsb.tile([18, NPOS], f32, tag="et")
                nc.scalar.activation(out=et[:], in_=qt[:], func=AF.Exp,
                                     scale=cs[0:18, _D2:_D2 + 1].bitcast(f32))
                graw = gsb.tile([18, NPOS], f32r, tag="graw")
                nc.vector.tensor_tensor(out=graw[:], in0=et[:], in1=inv[:], op=MULT)
                gb = gsb.tile([18, NPOS], bf16, tag="gb")
                nc.scalar.activation(out=gb[:], in_=graw[:].bitcast(f32), func=AF.Copy)

                # ---- normalizer path (hidden behind the unfold) ----
                S = ps_m.tile([2, NPOS], f32, tag="S")
                nc.tensor.matmul(S[:], gs[:, 9 * 128:9 * 128 + 2],
                                 gb[:], start=True, stop=True)
                Sc = gsb.tile([2, NPOS], f32r, tag="Sc")
                nc.vector.tensor_copy(out=Sc[:], in_=S[:])
                Srep = ps_m2.tile([128, NPOS], f32, tag="Srep")
                nc.tensor.matmul(Srep[:], cs[0:2, _OR:_OR + 128],
                                 Sc[:], start=True, stop=True)
                rr = gsb.tile([128, NPOS], f32, tag="rr")
                nc.vector.reciprocal_approx_fast(out=rr[:], in_=Srep[:])

                # ---- unfold: yt[tap] = grep_bf16(tap) * x_bf16, tree-sum, scale ----
                yt = y_p.tile([128, 9, BR, WO], bf16, tag="yt")
                for tap in range(9):
                    grep = ps_g.tile([128, NPOS], f32, tag="grep")
                    nc.tensor.matmul(grep[:], gs[:, tap * 128:(tap + 1) * 128],
                                     gb[:], start=True, stop=True)
                    gsb16 = y_p.tile([128, BR, WO], bf16, tag=f"gs{tap % 3}")
                    nc.scalar.activation(out=gsb16[:], in_=grep[:], func=AF.Copy)
                    eng = nc.gpsimd if tap % 3 == 2 else nc.vector
                    eng.tensor_tensor(out=yt[:, tap], in0=xtap_bf(tap, blk), in1=gsb16[:], op=MULT)
                t4 = y_p.tile([128, 4, BR, WO], bf16, tag="t4")
                nc.gpsimd.tensor_tensor(out=t4[:], in0=yt[:, 0:8:2], in1=yt[:, 1:8:2], op=ADD)
                t2 = y_p.tile([128, 2, BR, WO], bf16, tag="t2")
                nc.vector.tensor_tensor(out=t2[:], in0=t4[:, 0:4:2], in1=t4[:, 1:4:2], op=ADD)
                t1 = y_p.tile([128, BR, WO], f32, tag="t1")
                nc.vector.tensor_tensor(out=t1[:], in0=t2[:, 0], in1=t2[:, 1], op=ADD)
                t0 = y_p.tile([128, BR, WO], f32, tag="t0")
                nc.vector.tensor_tensor(out=t0[:], in0=t1[:], in1=yt[:, 8], op=ADD)
                acc = y_p.tile([128, BR, WO], f32, tag="acc")
                nc.gpsimd.tensor_tensor(out=acc[:], in0=t0[:], in1=rr[:], op=MULT)
                nc.sync.dma_start(out=out[:, BR * blk:BR * (blk + 1), :], in_=acc[:])

            sigs = {0: conv_emit(0)}
            for blk in range(NBLK):
                if blk + 1 < NBLK:
                    sigs[blk + 1] = conv_emit(blk + 1)
                block_emit(blk, sigs.pop(blk))

    if not for_sim and not nc.is_finalized():
        nc.finalize()
    return nc


def _prep_inputs(x, conv_w, bn_gamma, bn_beta, bn_mean, bn_var):
    cst = _build_consts(conv_w, bn_gamma, bn_beta, bn_mean, bn_var)
    xp = np.pad(np.asarray(x, np.float32), ((0, 0), (0, 0), (1, 1), (1, 1)), mode="reflect")
    import ml_dtypes
    gin = np.zeros((18, 9 * 128 + 2), ml_dtypes.bfloat16)
    for hh in range(HH):
        gin[hh * 9:hh * 9 + 9, 9 * 128 + hh] = 1.0
        for tap in range(9):
            gin[hh * 9 + tap, tap * 128 + hh * 64:tap * 128 + hh * 64 + 64] = 1.0
    in_maps = []
    for n in range(N):
        xc = np.concatenate([xp[n, :, 0:RS, :], xp[n, :, 64:64 + RS, :]], axis=0)
        in_maps.append({"xin": np.ascontiguousarray(xc), "cin": cst, "gin": gin})
    return in_maps


def _gather(results):
    out = np.empty((N, C, HO, WO), np.float32)
    for n in range(N):
        d = results[n]["out"]
        out[n, :, 0:HOC, :] = d[0:64]
        out[n, :, HOC:, :] = d[64:128]
    return out


def _enable_axon_trace():
    """Register the NTFF profile hook that this image's antenv lacks."""
    if _STATE.get("trace_hooked"):
        return
    import types
    import antenv
    from concourse import bass_utils
    mod = types.ModuleType("antenv.axon_hooks")
    mod._hook = None
    mod.set_axon_ntff_profile_hook = lambda h: setattr(mod, "_hook", h)
    mod.get_axon_ntff_profile_hook = lambda: mod._hook
    sys.modules["antenv.axon_hooks"] = mod
    antenv.axon_hooks = mod
    from trn_agent_boot.trn_boot import _ntff_profile_via_ctypes
    mod._hook = _ntff_profile_via_ctypes("/opt/axon/libaxon_pjrt.so")
    bass_utils.upload_artifacts = lambda tmpdir: tmpdir
    _STATE["trace_hooked"] = True


def run(x, conv_w, bn_gamma, bn_beta, bn_mean, bn_var, trace=False):
    from concourse.bass_utils import run_bass_kernel_spmd
    if trace:
        _enable_axon_trace()
    if "nc" not in _STATE:
        _STATE["nc"] = _build_bass()
    in_maps = _prep_inputs(x, conv_w, bn_gamma, bn_beta, bn_mean, bn_var)
    res = run_bass_kernel_spmd(_STATE["nc"], in_maps, list(range(N)), trace=trace)
    _STATE["last"] = res
    return _gather(res.results)


def kernel(x, conv_w, bn_gamma, bn_beta, bn_mean, bn_var):
    return run(x, conv_w, bn_gamma, bn_beta, bn_mean, bn_var,
               trace=bool(int(os.environ.get("KERNEL_TRACE", "0"))))


# revision 18
# speedup vs baseline: 1.3987x; 1.2169x over previous
"""Trainium2 Bass kernel for nn_Downsample_Spa: sigma-conv + gaussian unfold downsample.

Math (per batch image, all on one NeuronCore; batch of 8 -> 8 cores):
  xp = reflect_pad(x)                                  # [64,130,130]
  sigma[o,p] = clamp(BN(conv3x3(xp))[o,p], 1e-4)       # only needed at stride-2 positions p
  graw[o,p]  = exp(-0.5*d2[o]/sigma^2) / sigma         # (sqrt(2pi) cancels in normalization)
  gn[o,p]    = graw[o,p] / sum_o' graw[o',p]
  out[c,p]   = sum_o gn[o,p] * xp[c, p + offset(o)]

Device layout: partitions = (row-half hh, channel c) = 128. Host prepends
reflect padding so every tap is a clean strided AP. Conv runs as 9 accumulating
fp32r matmuls with block-diagonal weights (M=18 covers both row halves in one
N-stream). g pipeline runs on ACT via exp/ln (no reciprocal needed). The
per-position g is broadcast across the 64 channel partitions with a one-hot PE
matmul, then DVE multiplies and accumulates the 9 taps.
"""

import os
import sys

import numpy as np

if "/opt/trn_rl_repo" not in sys.path:
    sys.path.insert(0, "/opt/trn_rl_repo")

K = 3
BN_EPS = 1e-5
SIGMA_MIN = 1e-4
N, C, H, W = 8, 64, 128, 128
HO = WO = 64            # output spatial dims
HH = 2                  # row halves
RS = 65                 # padded-row slots per partition
WS = 130                # padded-col slots
HOC = 32                # output rows per half
NBLK = 4                # position blocks (8 output-row-pairs each)
BR = HOC // NBLK        # ho' rows per block = 8
NPOS = BR * WO          # matmul N per block = 512

# consts tensor column layout
_W0 = 0                  # Wblk: 9 taps x 18 cols
_OS = _W0 + 9 * 18       # onesS [18,2]
_OR = _OS + 2            # onesR [2,128]
_OG = _OR + 128          # onesG: 9 taps x 128 cols
_D2 = _OG + 9 * 128      # d2 scale [18,1]
_BC = _D2 + 1            # bias-clamp [18,1]
_EP = _BC + 1            # sigma_min const [18,1]
_NCC = _EP + 1

_STATE = {}


def _build_consts(conv_w, bn_gamma, bn_beta, bn_mean, bn_var):
    s = (bn_gamma / np.sqrt(bn_var + BN_EPS)).astype(np.float32)      # [9]
    wf = conv_w.astype(np.float32) * s[:, None, None, None]           # [9,64,3,3]
    bias = (bn_beta - bn_mean * s).astype(np.float32)                 # [9]

    cst = np.zeros((128, _NCC), np.float32)
    for tap in range(9):
        i, j = tap // 3, tap % 3
        for hh in range(HH):
            # lhsT column m = hh*9 + o ; row k = hh*64 + c
            cst[hh * 64:hh * 64 + 64, _W0 + tap * 18 + hh * 9:_W0 + tap * 18 + hh * 9 + 9] = \
                wf[:, :, i, j].T  # [c, o]
    for hh in range(HH):
        cst[hh * 9:hh * 9 + 9, _OS + hh] = 1.0                        # onesS [18,2]
        cst[hh, _OR + hh * 64:_OR + hh * 64 + 64] = 1.0              # onesR [2,128]
        for tap in range(9):
            cst[hh * 9 + tap, _OG + tap * 128 + hh * 64:_OG + tap * 128 + hh * 64 + 64] = 1.0
    d2 = np.array([(kk // 3 - 1) ** 2 + (kk % 3 - 1) ** 2 for kk in range(9)], np.float32)
    for hh in range(HH):
        cst[hh * 9:hh * 9 + 9, _D2] = -0.5 * d2
        cst[hh * 9:hh * 9 + 9, _BC] = bias - SIGMA_MIN
        cst[hh * 9:hh * 9 + 9, _EP] = SIGMA_MIN
    return cst


def _build_bass(for_sim=False):
    import concourse.bass as bass
    import concourse.tile as tile
    from concourse import mybir

    f32 = mybir.dt.float32
    f32r = mybir.dt.float32r
    bf16 = mybir.dt.bfloat16
    MULT = mybir.AluOpType.mult
    ADD = mybir.AluOpType.add
    MAX = mybir.AluOpType.max
    AF = mybir.ActivationFunctionType

    if for_sim:
        nc = bass.Bass("TRN2", target_bir_lowering=False, detect_race_conditions=False)
    else:
        from concourse import bacc
        nc = bacc.Bacc()
    xin = nc.dram_tensor("xin", [128, RS, WS], f32r, kind="ExternalInput")
    cin = nc.dram_tensor("cin", [128, _NCC], f32r, kind="ExternalInput")
    gin = nc.dram_tensor("gin", [18, 10 * 128], bf16, kind="ExternalInput")
    out = nc.dram_tensor("out", [128, HOC, WO], f32, kind="ExternalOutput")

    CR = 17                  # rows per chunk tile (16 + 1 overlap)
    JW = 66                  # parity-plane j slots (65 used + pad)

    with tile.TileContext(nc) as tc:
        from contextlib import ExitStack
        with ExitStack() as ctx:
            big = ctx.enter_context(tc.tile_pool(name="big", bufs=1))
            gsb = ctx.enter_context(tc.tile_pool(name="gsb", bufs=2))
            y_p = ctx.enter_context(tc.tile_pool(name="y", bufs=2))
            ps_s = ctx.enter_context(tc.tile_pool(name="ps_s", bufs=2, space="PSUM"))
            ps_m = ctx.enter_context(tc.tile_pool(name="ps_m", bufs=2, space="PSUM"))
            ps_m2 = ctx.enter_context(tc.tile_pool(name="ps_m2", bufs=2, space="PSUM"))
            ps_g = ctx.enter_context(tc.tile_pool(name="ps_g", bufs=2, space="PSUM"))

            cs = big.tile([128, _NCC], f32r)
            nc.sync.dma_start(out=cs[:], in_=cin[:])
            gs = big.tile([18, 10 * 128], bf16)
            nc.sync.dma_start(out=gs[:], in_=gin[:])

            # chunked input: per block a [128, 17, 130] f32r tile (+1 row overlap)
            xsk = []
            for blk in range(NBLK):
                xs = big.tile([128, CR, WS], f32r, tag=f"xs{blk}")
                nc.sync.dma_start(out=xs[:], in_=xin[:, 16 * blk:16 * blk + CR, :])
                xsk.append(xs)

            def xtap(tap, blk):
                # f32r strided view for the conv rhs
                i, j = tap // 3, tap % 3
                return xsk[blk][:, i:i + 2 * BR - 1:2, j:j + 2 * WO - 1:2]  # [128, 8, 64]

            def conv_emit(blk):
                sig = ps_s.tile([18, NPOS], f32, tag="sig")
                for tap in range(9):
                    nc.tensor.matmul(
                        sig[:],
                        cs[:, _W0 + tap * 18:_W0 + (tap + 1) * 18],
                        xtap(tap, blk),
                        start=(tap == 0), stop=(tap == 8),
                    )
                return sig

            def block_emit(blk, sig):
                # ---- g pipeline: graw = exp(-0.5*d2/sig^2)/sig (unnormalized) ----
                sc = gsb.tile([18, NPOS], f32, tag="sc")
                nc.vector.tensor_scalar(out=sc[:], in0=sig[:],
                                        scalar1=cs[0:18, _BC:_BC + 1].bitcast(f32),
                                        scalar2=float(SIGMA_MIN),
                                        op0=ADD, op1=MAX)
                inv = gsb.tile([18, NPOS], f32, tag="inv")
                nc.vector.reciprocal_approx_fast(out=inv[:], in_=sc[:])
                qt = gsb.tile([18, NPOS], f32, tag="qt")
                nc.scalar.activation(out=qt[:], in_=inv[:], func=AF.Square)
                et = gsb.tile([18, NPOS], f32, tag="et")
                nc.scalar.activation(out=et[:], in_=qt[:], func=AF.Exp,
                                     scale=cs[0:18, _D2:_D2 + 1].bitcast(f32))
                gb = gsb.tile([18, NPOS], bf16, tag="gb")
                nc.vector.tensor_tensor(out=gb[:], in0=et[:], in1=inv[:], op=MULT)

                # ---- normalizer = 10th tap: Srep[(hh,c),p] = sum_o gb[(hh,o),p] ----
                Srep = ps_m2.tile([128, NPOS], f32, tag="Srep")
                nc.tensor.matmul(Srep[:], gs[:, 9 * 128:9 * 128 + 128],
                                 gb[:], start=True, stop=True)
                rr = gsb.tile([128, NPOS], f32, tag="rr")
                nc.vector.reciprocal_approx_fast(out=rr[:], in_=Srep[:])

                # ---- unfold: yt[tap] = grep_bf16(tap) * x_bf16, tree-sum, scale ----
                # center tap (4) kept in fp32 (it carries ~|x| when sigma clamps);
                # 8 side taps in bf16 slots 0..7
                yt = y_p.tile([128, 8, BR, WO], bf16, tag="yt")
                y4 = y_p.tile([128, BR, WO], f32, tag="y4")
                for tap in range(9):
                    grep = ps_g.tile([128, NPOS], f32, tag="grep")
                    nc.tensor.matmul(grep[:], gs[:, tap * 128:(tap + 1) * 128],
                                     gb[:], start=True, stop=True)
                    dst = y4[:] if tap == 4 else yt[:, tap if tap < 4 else tap - 1]
                    nc.vector.tensor_tensor(out=dst, in0=xtap(tap, blk).bitcast(f32),
                                            in1=grep[:], op=MULT)
                t4 = y_p.tile([128, 4, BR, WO], bf16, tag="t4")
                nc.vector.tensor_tensor(out=t4[:], in0=yt[:, 0:8:2], in1=yt[:, 1:8:2], op=ADD)
                t2 = y_p.tile([128, 2, BR, WO], bf16, tag="t2")
                nc.vector.tensor_tensor(out=t2[:], in0=t4[:, 0:4:2], in1=t4[:, 1:4:2], op=ADD)
                t1 = y_p.tile([128, BR, WO], bf16, tag="t1")
                nc.gpsimd.tensor_tensor(out=t1[:], in0=t2[:, 0], in1=t2[:, 1], op=ADD)
                t0 = y_p.tile([128, BR, WO], f32, tag="t0")
                nc.gpsimd.tensor_tensor(out=t0[:], in0=t1[:], in1=y4[:], op=ADD)
                acc = y_p.tile([128, BR, WO], f32, tag="acc")
                nc.gpsimd.tensor_tensor(out=acc[:], in0=t0[:], in1=rr[:], op=MULT)
                nc.sync.dma_start(out=out[:, BR * blk:BR * (blk + 1), :], in_=acc[:])

            sigs = {0: conv_emit(0)}
            for blk in range(NBLK):
                if blk + 1 < NBLK:
                    sigs[blk + 1] = conv_emit(blk + 1)
                block_emit(blk, sigs.pop(blk))

    if not for_sim and not nc.is_finalized():
        nc.finalize()
    return nc


def _prep_inputs(x, conv_w, bn_gamma, bn_beta, bn_mean, bn_var):
    cst = _build_consts(conv_w, bn_gamma, bn_beta, bn_mean, bn_var)
    xp = np.pad(np.asarray(x, np.float32), ((0, 0), (0, 0), (1, 1), (1, 1)), mode="reflect")
    import ml_dtypes
    gin = np.zeros((18, 10 * 128), ml_dtypes.bfloat16)
    for hh in range(HH):
        gin[hh * 9:hh * 9 + 9, 9 * 128 + hh * 64:9 * 128 + hh * 64 + 64] = 1.0
        for tap in range(9):
            gin[hh * 9 + tap, tap * 128 + hh * 64:tap * 128 + hh * 64 + 64] = 1.0
    in_maps = []
    for n in range(N):
        xc = np.concatenate([xp[n, :, 0:RS, :], xp[n, :, 64:64 + RS, :]], axis=0)
        in_maps.append({"xin": np.ascontiguousarray(xc), "cin": cst, "gin": gin})
    return in_maps


def _gather(results):
    out = np.empty((N, C, HO, WO), np.float32)
    for n in range(N):
        d = results[n]["out"]
        out[n, :, 0:HOC, :] = d[0:64]
        out[n, :, HOC:, :] = d[64:128]
    return out


def _enable_axon_trace():
    """Register the NTFF profile hook that this image's antenv lacks."""
    if _STATE.get("trace_hooked"):
        return
    import types
    import antenv
    from concourse import bass_utils
    mod = types.ModuleType("antenv.axon_hooks")
    mod._hook = None
    mod.set_axon_ntff_profile_hook = lambda h: setattr(mod, "_hook", h)
    mod.get_axon_ntff_profile_hook = lambda: mod._hook
    sys.modules["antenv.axon_hooks"] = mod
    antenv.axon_hooks = mod
    from trn_agent_boot.trn_boot import _ntff_profile_via_ctypes
    mod._hook = _ntff_profile_via_ctypes("/opt/axon/libaxon_pjrt.so")
    bass_utils.upload_artifacts = lambda tmpdir: tmpdir
    _STATE["trace_hooked"] = True


def run(x, conv_w, bn_gamma, bn_beta, bn_mean, bn_var, trace=False):
    from concourse.bass_utils import run_bass_kernel_spmd
    if trace:
        _enable_axon_trace()
    if "nc" not in _STATE:
        _STATE["nc"] = _build_bass()
    in_maps = _prep_inputs(x, conv_w, bn_gamma, bn_beta, bn_mean, bn_var)
    res = run_bass_kernel_spmd(_STATE["nc"], in_maps, list(range(N)), trace=trace)
    _STATE["last"] = res
    return _gather(res.results)


def kernel(x, conv_w, bn_gamma, bn_beta, bn_mean, bn_var):
    return run(x, conv_w, bn_gamma, bn_beta, bn_mean, bn_var,
               trace=bool(int(os.environ.get("KERNEL_TRACE", "0"))))
